# revision 43
# baseline (speedup 1.0000x reference)
"""Trainium2 Bass kernel for nn_Autorec_DG_13116830122688 (AutoRec + GraphConv0D).

Math (reference):
    h   = sigmoid(x @ enc_w.T + enc_b)                      [N, 500]
    agg = segment_sum(h[src] * edge_weight, dst, N)
    hm  = conv_w * agg + (1 - conv_w) * h
    p   = clip(hm @ dec_w.T + dec_b, 1, 5)
    p   = where(ft_n0 == 0 rows, fill, p); where(ft_n1 == 0 cols, fill, p)

Strategy (8 NeuronCores, data-parallel over users):
  - Shard users 2500/core (padded to 2560 = 20x128 tiles).
  - Encoder: x is pre-transposed to item-major [128, KC*128] tiles ON HOST
    (bf16), so each user tile is one contiguous 1.5MB DMA and the 47-chunk
    matmul accumulation runs with no PE transposes.  Encoder bias folded in
    as an extra always-one input column.  ACT sigmoid -> h bf16 (SBUF
    resident for the whole kernel).  Decoder weights and gather indices are
    loaded AFTER the first x tile so the PE starts ~35us earlier.
  - AllGather h (bf16, only the 2500 real rows, 512-wide) in 10 chunks
    overlapped with the encoder so every core can gather any source
    embedding; single-tile chunks at the tail keep the serial collective
    stream draining right behind the encoder (last chunk = 68 rows).
  - Message passing: edges are filtered (masked-dst rows dropped), scaled by
    conv_w, self-loops with weight (1-conv_w) added, sorted by dst and packed
    into 128-edge blocks per 128-dst tile.  ONE gpsimd dma_gather per dst
    tile fetches all its source rows (sub-1us issue; int16 indices in the
    16-partition wrapped layout), then each block multiplies a host-built
    [128 edges x 128 dst] sparse weight matrix on the TensorEngine:
    aggT += G.T @ W accumulates in PSUM in hidden-major layout, which feeds
    the decoder with no extra transpose.  The self-loop block reads this
    core's h directly from SBUF (no DMA).
  - Decoder: p = hmT.T @ dec_w.T with the column mask and fill constant baked
    into host-prepped weights, plus two extra hidden units carrying the decoder
    bias and the row-mask fill. Single DVE instruction clips to [1, 5] and
    emits fp16 (upcast to f32 on host).  Decoder of tile t-1 is emitted after
    message matmuls of tile t so the PE never idles waiting on the hmT copy.
"""

import os
import sys

import numpy as np

for _p in ("/opt/trn_rl_repo",):
    if _p not in sys.path and os.path.isdir(_p):
        sys.path.insert(0, _p)

import ml_dtypes  # noqa: E402

# ---- problem constants (hardcoded per contest rules) ----
N_USERS = 20000
N_ITEMS = 6000
HIDDEN = 500
M = 8  # cores
UPC = N_USERS // M  # 2500 users per core
UT = 20  # user tiles per core
UPAD = UT * 128  # 2560
KC = 47  # item chunks of 128 (6016 = 47*128 >= 6001 incl. bias col)
IPAD = KC * 128  # 6016
HPAD = 504  # hidden padded: 4 chunks of 126 (500 real + bias/mask units)
NCH = 12  # decoder output chunks of 500 (12*500 = 6000)
R_MIN, R_MAX = 1.0, 5.0
# all-gather chunk boundaries in user tiles (cumulative).  The collective
# runs as a serial stream (~9us/tile transfer + ~5us fixed per chunk, and a
# ~17us fixed cost on the FINAL chunk regardless of its size), so few large
# chunks beat many small ones; measured end ~307us with these bounds.
CC_TILE_BOUNDS = [3, 7, 11, 15, 18, 20]
# Every dst tile's gather is split into a PREFIX window (h_full rows written
# by chunks 0..3, i.e. local rows < 1920 of every core, ~77% of edges) and a
# SUFFIX window (the last two chunks).  Prefix gathers only wait on chunk 3
# (lands ~257us, well before the encoder ends), so their ~7us serial gpsimd
# issues and executions all run during the all-gather tail; KPRE of them are
# emitted ahead of the msg loop.  Suffix-free tiles 0..NSFREE-1 (see the
# host permutation) keep the PE decoding through the final chunk's landing.
KPRE = 2
NSFREE = 3

_bf16 = ml_dtypes.bfloat16

_PROGRAM_CACHE = {}


def _build_program(S_PRE, S_SUF):
    """Build the SPMD Bass program.

    Every dst tile's message pass reads two gather windows: PREFIX (h_full
    rows < PFX, i.e. all all-gather chunks but the last) and SUFFIX (the
    final chunk's rows).  Both feed one contiguous PSUM accumulation group
    together with the SBUF self-loop block.
    """
    import concourse.bass as bass
    import concourse.bacc as bacc
    import concourse.mybir as mybir
    from concourse import library_config
    from concourse.tile import TileContext

    P = 128
    f32 = mybir.dt.float32
    f16 = mybir.dt.float16
    bf16 = mybir.dt.bfloat16
    NBLK_PRE = sum(S_PRE)
    NBLK = NBLK_PRE + sum(S_SUF)
    BOFF_PRE = [sum(S_PRE[:t]) for t in range(UT)]
    BOFF_SUF = [NBLK_PRE + sum(S_SUF[:t]) for t in range(UT)]

    nc = bacc.Bacc(
        "TRN2",
        target_bir_lowering=False,
        debug=False,
        num_devices=M,
        num_swdge_queues=4,
    )

    # x pre-transposed on host: row ut*128+p (item-in-chunk), col k*128+u
    x_d = nc.declare_dram_parameter("x", [UPAD, KC * P], bf16, isOutput=False)
    encw_d = nc.declare_dram_parameter("encw", [P, KC * HIDDEN], bf16, isOutput=False)
    decw_d = nc.declare_dram_parameter("decw", [P, 4 * N_ITEMS], bf16, isOutput=False)
    i16 = mybir.dt.int16
    # gather indices for dma_gather: idx j of tile t at column boff[t]*8 +
    # j//16, partition j%16, replicated 8x down the 128 partitions.
    si_d = nc.declare_dram_parameter("sidx", [P, NBLK * 8], i16, isOutput=False)
    wb_d = nc.declare_dram_parameter("wblk", [NBLK + UT, P, P], bf16, isOutput=False)
    rv_d = nc.declare_dram_parameter("rowvec", [4, UPAD], bf16, isOutput=False)
    out_d = nc.declare_dram_parameter("out", [UPC, N_ITEMS], f16, isOutput=True)

    # only the 2500 real rows travel through the all-gather.  Rows are 512
    # wide (dma_gather needs a 256B-multiple row stride); cols 500-511 are
    # never written — the garbage lands only in hmT rows that the rv DMA
    # overwrites (hidden 500+) or that carry zero decoder weight.
    HW = 512
    h_loc = nc.dram_tensor("h_loc", [UPC, HW], bf16)
    h_full = nc.dram_tensor("h_full", [M * UPC, HW], bf16, addr_space="Shared")
    CC_ROW_BOUNDS = [min(b * 128, UPC) for b in CC_TILE_BOUNDS]
    PFX = M * CC_ROW_BOUNDS[-3]  # prefix rows: chunks 0..3

    with TileContext(nc) as tc:
        with (
            tc.tile_pool(name="const", bufs=1) as cpool,
            tc.tile_pool(name="xin", bufs=2) as xpool,
            tc.tile_pool(name="hsb", bufs=UT) as hpool,
            # bufs=2 doubles as a THROTTLE: pre(t+2)'s gather reuses pre(t)'s
            # buffer, so it cannot execute until msg(t) consumed it — at most
            # ~2 gathers' random reads ever collide with the in-flight
            # all-gather chunks (measured: unthrottled gathers stretched a
            # 35us chunk to 64us).
            tc.tile_pool(name="gpre", bufs=2) as prepool,
            tc.tile_pool(name="gsuf", bufs=2) as sufpool,
            tc.tile_pool(name="wbl", bufs=4 * max(S_PRE) + 10) as wpool,
            tc.tile_pool(name="hmt", bufs=4) as mpool,
            tc.tile_pool(name="pout", bufs=2) as opool,
            tc.tile_pool(name="ps_acc", bufs=3, space="PSUM") as ps_acc,
            tc.tile_pool(name="ps_dec", bufs=5, space="PSUM") as ps_dec,
        ):
            # dma_gather (InstDMAGatherAnt) lives in the gpsimd mlp library;
            # load it before any gpsimd instruction.
            nc.gpsimd.load_library(library_config.mlp)

            # encoder weights split into pieces so tile 0's matmuls start
            # after a few hundred KB (first pieces) instead of 7.5MB.
            enc_sb = cpool.tile([P, KC * HIDDEN], bf16, tag="encw")
            ENC_SPLIT = [6, 18, 32, KC]
            nc.sync.dma_start(
                out=enc_sb[:, : ENC_SPLIT[0] * HIDDEN],
                in_=encw_d[:, : ENC_SPLIT[0] * HIDDEN],
            )
            si_sb = cpool.tile([P, NBLK * 8], i16, tag="sidx")
            dec_sb = cpool.tile([P, 4 * N_ITEMS], bf16, tag="decw")

            # ---------------- Phase 1: encoder ----------------
            def load_x(ut):
                xb = xpool.tile([P, KC * P], bf16, tag="xb", name=f"xb{ut}")
                nc.sync.dma_start(out=xb[:], in_=x_d[ut * P : (ut + 1) * P, :])
                return xb

            hsbs = []
            xb_next = load_x(0)
            for ut in range(UT):
                xb = xb_next
                if ut == 0:
                    # interleave the remaining encoder-weight pieces with the
                    # x1 load so neither starves the tile-0/1 matmul chain.
                    nc.sync.dma_start(
                        out=enc_sb[:, ENC_SPLIT[0] * HIDDEN : ENC_SPLIT[1] * HIDDEN],
                        in_=encw_d[:, ENC_SPLIT[0] * HIDDEN : ENC_SPLIT[1] * HIDDEN],
                    )
                    xb_next = load_x(1)
                    e_lo = ENC_SPLIT[1]
                    for e_hi in ENC_SPLIT[2:]:
                        nc.sync.dma_start(
                            out=enc_sb[:, e_lo * HIDDEN : e_hi * HIDDEN],
                            in_=encw_d[:, e_lo * HIDDEN : e_hi * HIDDEN],
                        )
                        e_lo = e_hi
                elif ut + 1 < UT:
                    xb_next = load_x(ut + 1)
                h_ps = ps_acc.tile([P, 512], f32, tag="acc")
                for k in range(KC):
                    nc.tensor.matmul(
                        out=h_ps[:, :HIDDEN],
                        lhsT=xb[:, k * P : (k + 1) * P],
                        rhs=enc_sb[:, k * HIDDEN : (k + 1) * HIDDEN],
                        start=(k == 0),
                        stop=(k == KC - 1),
                    )
                # bufs=UT and exactly UT allocations of this tag: every tile
                # keeps its own SBUF-resident buffer for the whole kernel.
                hsb = hpool.tile([P, HPAD], bf16, tag="hsb")
                hsbs.append(hsb)
                nc.scalar.activation(
                    out=hsb[:, :HIDDEN],
                    in_=h_ps[:, :HIDDEN],
                    func=mybir.ActivationFunctionType.Sigmoid,
                )
                nc.vector.memset(hsb[:, HIDDEN:HPAD], 0.0)
                nr = min((ut + 1) * P, UPC) - ut * P  # 68 real rows on tile 19
                nc.sync.dma_start(
                    out=h_loc[ut * P : ut * P + nr, :HIDDEN], in_=hsb[:nr, :HIDDEN]
                )
                if ut == 2:
                    # deferred const loads: issued after the first x tiles so
                    # the encoder pipeline fills before they take bandwidth.
                    nc.sync.dma_start(out=si_sb[:], in_=si_d[:])
                elif ut in (4, 7, 10, 13):
                    # decoder weights in 1.5MB quarters spread across the
                    # encoder so no single load starves the x-tile stream.
                    j = (4, 7, 10, 13).index(ut)
                    nc.sync.dma_start(
                        out=dec_sb[:, j * N_ITEMS : (j + 1) * N_ITEMS],
                        in_=decw_d[:, j * N_ITEMS : (j + 1) * N_ITEMS],
                    )
                # ---- Phase 2 (interleaved): chunked all-gather ----
                if (ut + 1) in CC_TILE_BOUNDS:
                    j = CC_TILE_BOUNDS.index(ut + 1)
                    rlo = 0 if j == 0 else CC_ROW_BOUNDS[j - 1]
                    rhi = CC_ROW_BOUNDS[j]
                    nc.gpsimd.collective_compute(
                        "AllGather",
                        mybir.AluOpType.bypass,
                        replica_groups=[list(range(M))],
                        ins=[h_loc[rlo:rhi, :]],
                        outs=[h_full[M * rlo : M * rhi, :]],
                    )

            # ---------------- Phase 3: message passing + decoder ----------------
            # Software-pipelined: message matmuls of tile t are emitted before
            # decoder matmuls of tile t-1, so the PE stays busy while ACT
            # drains agg(t) into hmT(t).
            hmTs = [None] * UT
            GTW_PRE = max([1] + list(S_PRE))
            GTW_SUF = max([1] + list(S_SUF))
            pres = [None] * UT
            sufs = [None] * UT

            def gather(St, boff, lo, hi, q, pool, W):
                # ONE dma_gather fetches all St*128 source rows: out[p, s, :]
                # = h_full[lo + idx[s*128+p]].  The sliced source AP keeps the
                # dependency on just the all-gather chunks covering [lo, hi).
                gt_all = pool.tile([P, W, HW], bf16, tag="gt")
                nc.gpsimd.dma_gather(
                    gt_all[:, :St, :],
                    h_full[lo:hi, :],
                    si_sb[:, boff * 8 : (boff + St) * 8],
                    St * P,
                    St * P,
                    HW,
                    queue_num=q,
                )
                return gt_all

            def emit_pre(t):
                # prefix waits only on all-gather chunks 0..3, so its serial
                # gpsimd issue overlaps the collective tail.
                if S_PRE[t]:
                    pres[t] = gather(
                        S_PRE[t], BOFF_PRE[t], 0, PFX, t % 4, prepool, GTW_PRE
                    )

            def emit_suf(t):
                # suffix waits on the final all-gather chunk
                if S_SUF[t]:
                    sufs[t] = gather(
                        S_SUF[t], BOFF_SUF[t], PFX, M * UPC, (t + 2) % 4,
                        sufpool, GTW_SUF,
                    )

            def load_wbs(boff, n):
                wbs = []
                for s in range(n):
                    wb = wpool.tile([P, P], bf16, tag="wb")
                    nc.sync.dma_start(out=wb[:], in_=wb_d[boff + s])
                    wbs.append(wb)
                return wbs

            def emit_msg_core(t, agg_ps):
                # drain agg psum -> hmT bf16 (+ bias/fill rows via DMA)
                hmT = mpool.tile([P, 512], bf16, tag="hmT")
                hmTs[t] = hmT
                # hidden unit 500 (chunk 3, row 122): decoder-bias unit
                # hidden unit 501 (chunk 3, row 123): row-mask fill unit
                # rows 124-125 are zero padding.  The ACT copy below skips
                # rows 122+ of chunk 3, so this DMA has no dependency on the
                # agg drain and can land during the message matmuls.
                nc.sync.dma_start(
                    out=hmT[122:126, 3 * P : 4 * P],
                    in_=rv_d[0:4, t * P : (t + 1) * P],
                )
                nc.scalar.activation(
                    out=hmT[0:126, 0 : 3 * P],
                    in_=agg_ps[0:126, 0 : 3 * P],
                    func=mybir.ActivationFunctionType.Copy,
                )
                nc.scalar.activation(
                    out=hmT[0:122, 3 * P : 4 * P],
                    in_=agg_ps[0:122, 3 * P : 4 * P],
                    func=mybir.ActivationFunctionType.Copy,
                )

            def emit_msg(t):
                agg_ps = ps_acc.tile([P, 512], f32, tag="acc")
                Sp, Ss = S_PRE[t], S_SUF[t]
                wbs = load_wbs(BOFF_PRE[t], Sp)
                wbs += load_wbs(BOFF_SUF[t], Ss)
                # self-loop block: this core's own h tile straight from SBUF
                # with a diagonal weight block (no gather, no DMA).
                wbself = wpool.tile([P, P], bf16, tag="wb")
                nc.sync.dma_start(out=wbself[:], in_=wb_d[NBLK + t])
                wbs.append(wbself)
                nblk = Sp + Ss  # + 1 self block
                # keep each PSUM sub-region's accumulation group contiguous:
                # interleaved start=True matmuls in one bank clobber each
                # other's accumulation state.
                for c in range(4):
                    for s in range(nblk + 1):
                        if s < Sp:
                            lhsT = pres[t][:, s, c * 126 : (c + 1) * 126]
                        elif s < nblk:
                            lhsT = sufs[t][:, s - Sp, c * 126 : (c + 1) * 126]
                        else:
                            lhsT = hsbs[t][:, c * 126 : (c + 1) * 126]
                        nc.tensor.matmul(
                            out=agg_ps[0:126, c * P : (c + 1) * P],
                            lhsT=lhsT,
                            rhs=wbs[s][:],
                            start=(s == 0),
                            stop=(s == nblk),
                        )
                emit_msg_core(t, agg_ps)

            def emit_dec(t):
                hmT = hmTs[t]
                nu = UPC - t * P if t == UT - 1 else P  # 68 on the last tile
                for half in range(2):
                    # batch 6 x 500-col chunks into one SBUF row-block so the
                    # output DMA moves contiguous 6KB rows.
                    psb = opool.tile([P, 3000], f16, tag="psb")
                    for nn in range(6):
                        n = half * 6 + nn
                        p_ps = ps_dec.tile([P, 512], f32, tag="pps")
                        for c in range(4):
                            nc.tensor.matmul(
                                out=p_ps[:, :500],
                                lhsT=hmT[0:126, c * P : (c + 1) * P],
                                rhs=dec_sb[0:126, c * N_ITEMS + n * 500 : c * N_ITEMS + (n + 1) * 500],
                                start=(c == 0),
                                stop=(c == 3),
                            )
                        nc.vector.tensor_scalar(
                            out=psb[:, nn * 500 : (nn + 1) * 500],
                            in0=p_ps[:, :500],
                            scalar1=R_MAX,
                            scalar2=R_MIN,
                            op0=mybir.AluOpType.min,
                            op1=mybir.AluOpType.max,
                        )
                        if t == UT - 1 and nn % 2 == 1:
                            # last tile: drain in 1000-col pieces right behind
                            # the DVE so the kernel tail is one small DMA.
                            nc.sync.dma_start(
                                out=out_d[
                                    t * P : t * P + nu,
                                    half * 3000 + (nn - 1) * 500 : half * 3000 + (nn + 1) * 500,
                                ],
                                in_=psb[:nu, (nn - 1) * 500 : (nn + 1) * 500],
                            )
                    if t != UT - 1:
                        nc.sync.dma_start(
                            out=out_d[t * P : t * P + nu, half * 3000 : (half + 1) * 3000],
                            in_=psb[:nu, :],
                        )

            # KPRE tiles' prefix gathers are emitted ahead so their serial
            # gpsimd issues overlap the all-gather tail; suffix gathers come
            # after ALL primed prefixes (a suffix waits on the final chunk,
            # and gpsimd executes in order — a waiting suffix must not block
            # prefix issues).  Then the usual msg(t) / dec(t-1) interleave
            # with gathers staying KPRE tiles ahead.
            for t in range(KPRE):
                emit_pre(t)
            for t in range(KPRE):
                emit_suf(t)
            for t in range(UT):
                emit_msg(t)
                if t + KPRE < UT:
                    emit_pre(t + KPRE)
                    emit_suf(t + KPRE)
                if t > 0:
                    emit_dec(t - 1)
            emit_dec(UT - 1)

    nc.finalize()
    return nc


def _prep_host(x, edge_index, edge_weight, ft_n0, ft_n1, fill_const,
               enc_w, enc_b, dec_w, dec_b, conv_w):
    """All host-side preprocessing: sharding, weight prep, edge packing."""
    x = np.asarray(x, np.float32)
    src = np.asarray(edge_index[0], np.int64)
    dst = np.asarray(edge_index[1], np.int64)
    w = np.asarray(edge_weight, np.float32)
    ft_n0 = np.asarray(ft_n0)
    ft_n1 = np.asarray(ft_n1)
    fill = float(np.asarray(fill_const))
    conv = float(np.asarray(conv_w))
    enc_w = np.asarray(enc_w, np.float32)
    enc_b = np.asarray(enc_b, np.float32)
    dec_w = np.asarray(dec_w, np.float32)
    dec_b = np.asarray(dec_b, np.float32)

    # ---- user permutation: make dst tiles 0..NSFREE-1 suffix-free ----
    # Row order within a core is ours to choose (the output rows are
    # un-permuted on the host afterwards, costing no device time).  Swap
    # users so that no source sitting in the suffix rows (local rows >=
    # 1920, the last two all-gather chunks) has an edge into dst tiles
    # 0..NSFREE-1 of any core: those tiles' msg/dec then complete from
    # prefix+self alone and keep the PE busy through the final chunks'
    # landing (best-effort — leftover bad users degrade gracefully into a
    # small suffix block).
    SFX_LO = CC_TILE_BOUNDS[-3] * 128  # 1920
    E0 = NSFREE * 128
    src0 = np.asarray(edge_index[0], np.int64)
    dst0 = np.asarray(edge_index[1], np.int64)
    # bad2: users with an edge into dst rows < E0 (tiles 0..NSFREE-1);
    # bad1: the strict subset hitting tiles 0..1 — those swaps MUST happen
    # (a leftover would stall msg0/msg1 on the final chunk); the rest are
    # best-effort.
    bad2 = np.zeros(N_USERS, bool)
    bad2[src0[(dst0 % UPC) < E0]] = True
    bad1 = np.zeros(N_USERS, bool)
    bad1[src0[(dst0 % UPC) < 2 * 128]] = True
    perm = np.arange(N_USERS)  # position -> user
    for c in range(M):
        u0 = c * UPC
        late_pos = np.arange(u0 + SFX_LO, u0 + UPC)
        need_strict = late_pos[bad1[perm[late_pos]]]
        need_soft = late_pos[bad2[perm[late_pos]] & ~bad1[perm[late_pos]]]
        mid_pos = np.arange(u0 + E0, u0 + SFX_LO)
        pool = mid_pos[~bad2[perm[mid_pos]]]
        assert len(pool) >= len(need_strict), (len(pool), len(need_strict))
        need = np.concatenate([need_strict, need_soft])[: len(pool)]
        pool = pool[: len(need)]
        a = perm[need].copy()
        perm[need] = perm[pool]
        perm[pool] = a
    pos_of = np.empty(N_USERS, np.int64)
    pos_of[perm] = np.arange(N_USERS)

    x = x[perm]
    ft_n0 = np.asarray(ft_n0)[perm]
    src = pos_of[src]
    dst = pos_of[dst]

    rowmask = ft_n0 == 0  # rows forced to fill
    colmask = ft_n1 == 0  # cols forced to fill

    # ---- x per core, transposed to item-major tiles on host ----
    # layout: [UT, 128 (item-in-chunk p), KC, 128 (user u)] so each user
    # tile is one contiguous [128, KC*128] bf16 DMA and lhsT chunks are
    # direct column slices.
    xp = np.zeros((M, UPAD, IPAD), np.float32)
    xp[:, :UPC, :N_ITEMS] = x.reshape(M, UPC, N_ITEMS)
    xp[:, :, N_ITEMS] = 1.0  # encoder-bias input column
    xt_host = np.ascontiguousarray(
        xp.reshape(M, UT, 128, KC, 128).transpose(0, 1, 4, 3, 2)
    ).astype(_bf16).reshape(M, UPAD, KC * 128)

    # ---- encoder weights: [6016, 500] -> [128, 47*500] chunk-major ----
    ewp = np.zeros((IPAD, HIDDEN), np.float32)
    ewp[:N_ITEMS] = enc_w.T
    ewp[N_ITEMS] = enc_b
    enc_host = np.ascontiguousarray(
        ewp.reshape(KC, 128, HIDDEN).transpose(1, 0, 2).reshape(128, KC * HIDDEN)
    ).astype(_bf16)

    # ---- decoder weights with baked column mask / bias / fill units ----
    dw = dec_w.T.copy()  # [500, 6000]
    dw[:, colmask] = 0.0
    hp = np.zeros((HPAD, N_ITEMS), np.float32)
    hp[:HIDDEN] = dw
    hp[HIDDEN] = np.where(colmask, fill, dec_b)  # bias unit
    hp[HIDDEN + 1] = fill  # row-mask fill unit (all cols)
    dec_host = np.zeros((128, 4, N_ITEMS), np.float32)
    dec_host[:126] = hp.reshape(4, 126, N_ITEMS).transpose(1, 0, 2)
    dec_host = np.ascontiguousarray(dec_host.reshape(128, 4 * N_ITEMS)).astype(_bf16)

    # ---- edges: filter masked dst, fold conv_w ----
    keep = ~rowmask[dst]
    src_a = src[keep]
    dst_a = dst[keep]
    w_a = w[keep] * conv

    order = np.argsort(dst_a, kind="stable")
    src_a, dst_a, w_a = src_a[order], dst_a[order], w_a[order]

    core = dst_a // UPC
    ldst = dst_a - core * UPC
    tile_g = core * UT + ldst // 128  # global tile id (sorted ascending)
    din = (ldst % 128).astype(np.int64)

    # gather index into the PADDED all-gathered h table.
    # h_full layout after the uneven chunked all-gather: chunk j covers local
    # rows [lo_j*128, hi_j*128) of every core, concatenated core-major:
    # row = off_j + core * crows_j + (local - lo_j*128)
    src_core = src_a // UPC
    src_loc = src_a % UPC
    bounds_rows = np.array([min(b * 128, UPC) for b in CC_TILE_BOUNDS])
    starts_rows = np.concatenate([[0], bounds_rows[:-1]])
    crows = bounds_rows - starts_rows
    offs = np.concatenate([[0], np.cumsum(M * crows)[:-1]])
    cjs = np.searchsorted(bounds_rows, src_loc, side="right")
    gsrc_e = (
        offs[cjs] + src_core * crows[cjs] + (src_loc - starts_rows[cjs])
    ).astype(np.int64)

    # per-(tile, window) block quotas (max over cores, so the SPMD program is
    # identical on every core).  Every tile is split into a PREFIX window
    # (sources in h_full rows written by chunks 0..3, idx as-is) and a
    # SUFFIX window (last two chunks' rows, idx rebased) so the prefix
    # gathers only wait on all-gather chunk 3.
    PFX = M * int(bounds_rows[-3])
    in_sfx = gsrc_e >= PFX

    def pack(sel, rebase, min1_from=None):
        """Pack selected edges into per-tile 128-edge blocks.

        dma_gather index layout: idx j of tile t at column boff[t]*8 + j//16,
        partition j%16, replicated 8x down the 128 partitions.  Padding uses
        index 0 (gathers a real row, multiplied by weight 0).
        """
        tg = tile_g[sel]
        gi_all = gsrc_e[sel] - rebase
        dn = din[sel]
        ww = w_a[sel]
        cnt = np.bincount(tg, minlength=M * UT).reshape(M, UT)
        S_t = np.ceil(cnt.max(axis=0) / 128).astype(np.int64)
        if min1_from is not None:
            S_t[min1_from:] = np.maximum(1, S_t[min1_from:])
        boff = np.concatenate([[0], np.cumsum(S_t)[:-1]])
        nblk = int(S_t.sum())
        si_h = np.zeros((M, 128, nblk * 8), np.int16)
        wb_h = np.zeros((M, nblk, 128, 128), np.float32)
        starts = np.zeros(M * UT + 1, np.int64)
        np.cumsum(cnt.reshape(-1), out=starts[1:])
        for g in range(M * UT):
            c, t = divmod(g, UT)
            St = int(S_t[t])
            if St == 0:
                continue
            n = int(cnt[c, t])
            sl = slice(starts[g], starts[g] + n)
            cap = St * 128
            gi = np.zeros(cap, np.int64)
            wi = np.zeros(cap, np.float32)
            di = np.zeros(cap, np.int64)
            gi[:n] = gi_all[sl]
            wi[:n] = ww[sl]
            di[:n] = dn[sl]
            b0 = int(boff[t])
            wrap = gi.astype(np.int16).reshape(-1, 16).T  # [16, S*8]
            si_h[c, :, b0 * 8 : (b0 + St) * 8] = np.tile(wrap, (8, 1))
            for q in range(St):
                blk = slice(q * 128, (q + 1) * 128)
                wb_h[c, b0 + q][np.arange(128), di[blk]] = wi[blk]
        return S_t, si_h, wb_h

    SP_f, si_P, wb_P = pack(~in_sfx, 0)
    SS_f, si_S, wb_S = pack(in_sfx, PFX)
    S_PRE = tuple(int(v) for v in SP_f)
    S_SUF = tuple(int(v) for v in SS_f)
    si_host = np.concatenate([si_P, si_S], axis=2)
    wblk_host = np.concatenate([wb_P, wb_S], axis=1)
    # diagonal self-loop weight blocks, appended after the gather blocks:
    # block NBLK + t applies (1-conv)*live(d) to the SBUF h tile t.
    lv = np.zeros((M, UPAD), np.float32)
    lv[:, :UPC] = (~rowmask).reshape(M, UPC).astype(np.float32) * (1.0 - conv)
    wself = np.zeros((M, UT, 128, 128), np.float32)
    di128 = np.arange(128)
    for t in range(UT):
        wself[:, t, di128, di128] = lv[:, t * 128 : (t + 1) * 128]
    wblk_host = np.concatenate([wblk_host, wself], axis=1).astype(_bf16)

    # ---- row vectors: bias-unit coeff and row-mask coeff per padded user
    # (rows 2-3 are zero fillers for hmT pad rows 124-125) ----
    rv = np.zeros((M, 4, UPAD), np.float32)
    rm = rowmask.reshape(M, UPC)
    rv[:, 0, :UPC] = (~rm).astype(np.float32)  # bias unit on for live rows
    rv[:, 1, :UPC] = rm.astype(np.float32)     # fill unit on for masked rows
    rv_host = rv.astype(_bf16)

    in_maps = []
    for c in range(M):
        in_maps.append(
            {
                "x": xt_host[c],
                "encw": enc_host,
                "decw": dec_host,
                "sidx": si_host[c],
                "wblk": wblk_host[c],
                "rowvec": rv_host[c],
            }
        )
    return S_PRE, S_SUF, pos_of, in_maps


def _install_ntff_hook_shim():
    """The agent image's antenv lacks axon_hooks; synthesize it so
    run_bass_kernel_spmd(trace=True) can capture NTFF profiles."""
    import types

    if "antenv.axon_hooks" in sys.modules:
        return
    try:
        from trn_agent_boot.trn_boot import _ntff_profile_via_ctypes
    except ImportError:
        return
    hook = _ntff_profile_via_ctypes("/opt/axon/libaxon_pjrt.so")
    mod = types.ModuleType("antenv.axon_hooks")
    mod._hook = hook
    mod.set_axon_ntff_profile_hook = lambda h: setattr(mod, "_hook", h)
    mod.get_axon_ntff_profile_hook = lambda: mod._hook
    sys.modules["antenv.axon_hooks"] = mod
    try:
        import antenv

        antenv.axon_hooks = mod
    except ImportError:
        pass


LAST_EXEC_NS = None
LAST_RESULTS = None


def kernel(x, edge_index, edge_weight, ft_n0, ft_n1, fill_const,
           enc_w, enc_b, dec_w, dec_b, conv_w):
    global LAST_EXEC_NS, LAST_RESULTS
    from concourse.bass_utils import run_bass_kernel_spmd

    S_PRE, S_SUF, pos_of, in_maps = _prep_host(
        x, edge_index, edge_weight, ft_n0, ft_n1, fill_const,
        enc_w, enc_b, dec_w, dec_b, conv_w,
    )

    key = (S_PRE, S_SUF)
    if key not in _PROGRAM_CACHE:
        _PROGRAM_CACHE[key] = _build_program(S_PRE, S_SUF)
    nc = _PROGRAM_CACHE[key]

    trace = os.environ.get("KERNEL_TRACE", "0") == "1"
    tmpdir = os.environ.get("KERNEL_TRACE_DIR") or None
    if trace:
        _install_ntff_hook_shim()
    res = run_bass_kernel_spmd(
        nc,
        in_maps,
        core_ids=list(range(M)),
        trace=trace,
        tmpdir=tmpdir,
    )
    LAST_EXEC_NS = res.exec_time_ns
    LAST_RESULTS = res
    out = np.concatenate([res.results[c]["out"] for c in range(M)], axis=0)
    # rows come back in permuted (position) order; row for user u is at
    # position pos_of[u]
    return np.ascontiguousarray(out[pos_of].astype(np.float32))



# revision 49
# speedup vs baseline: 1.0323x; 1.0323x over previous
"""Trainium2 Bass kernel for nn_Autorec_DG_13116830122688 (AutoRec + GraphConv0D).

Math (reference):
    h   = sigmoid(x @ enc_w.T + enc_b)                      [N, 500]
    agg = segment_sum(h[src] * edge_weight, dst, N)
    hm  = conv_w * agg + (1 - conv_w) * h
    p   = clip(hm @ dec_w.T + dec_b, 1, 5)
    p   = where(ft_n0 == 0 rows, fill, p); where(ft_n1 == 0 cols, fill, p)

Strategy (8 NeuronCores, data-parallel over users):
  - Shard users 2500/core (padded to 2560 = 20x128 tiles).
  - Encoder: x is pre-transposed to item-major [128, KC*128] tiles ON HOST
    (bf16), so each user tile is one contiguous 1.5MB DMA and the 47-chunk
    matmul accumulation runs with no PE transposes.  Encoder bias folded in
    as an extra always-one input column.  ACT sigmoid -> h bf16 (SBUF
    resident for the whole kernel).  Decoder weights and gather indices are
    loaded AFTER the first x tile so the PE starts ~35us earlier.
  - AllGather h (bf16, only the 2500 real rows, 512-wide) in 10 chunks
    overlapped with the encoder so every core can gather any source
    embedding; single-tile chunks at the tail keep the serial collective
    stream draining right behind the encoder (last chunk = 68 rows).
  - Message passing: edges are filtered (masked-dst rows dropped), scaled by
    conv_w, self-loops with weight (1-conv_w) added, sorted by dst and packed
    into 128-edge blocks per 128-dst tile.  ONE gpsimd dma_gather per dst
    tile fetches all its source rows (sub-1us issue; int16 indices in the
    16-partition wrapped layout), then each block multiplies a host-built
    [128 edges x 128 dst] sparse weight matrix on the TensorEngine:
    aggT += G.T @ W accumulates in PSUM in hidden-major layout, which feeds
    the decoder with no extra transpose.  The self-loop block reads this
    core's h directly from SBUF (no DMA).
  - Decoder: p = hmT.T @ dec_w.T with the column mask and fill constant baked
    into host-prepped weights, plus two extra hidden units carrying the decoder
    bias and the row-mask fill. Single DVE instruction clips to [1, 5] and
    emits fp16 (upcast to f32 on host).  Decoder of tile t-1 is emitted after
    message matmuls of tile t so the PE never idles waiting on the hmT copy.
"""

import os
import sys

import numpy as np

for _p in ("/opt/trn_rl_repo",):
    if _p not in sys.path and os.path.isdir(_p):
        sys.path.insert(0, _p)

import ml_dtypes  # noqa: E402

# ---- problem constants (hardcoded per contest rules) ----
N_USERS = 20000
N_ITEMS = 6000
HIDDEN = 500
M = 8  # cores
UPC = N_USERS // M  # 2500 users per core
UT = 20  # user tiles per core
UPAD = UT * 128  # 2560
KC = 47  # item chunks of 128 (6016 = 47*128 >= 6001 incl. bias col)
IPAD = KC * 128  # 6016
HPAD = 504  # hidden padded: 4 chunks of 126 (500 real + bias/mask units)
NCH = 12  # decoder output chunks of 500 (12*500 = 6000)
R_MIN, R_MAX = 1.0, 5.0
# all-gather chunk boundaries in user tiles (cumulative).  The collective
# runs as a serial stream (~9us/tile transfer + ~5us fixed per chunk, and a
# ~17us fixed cost on the FINAL chunk regardless of its size), so few large
# chunks beat many small ones; measured end ~307us with these bounds.
CC_TILE_BOUNDS = [3, 7, 11, 15, 18, 20]
# Every dst tile's gather is split into a PREFIX window (h_full rows written
# by chunks 0..3, i.e. local rows < 1920 of every core, ~77% of edges) and a
# SUFFIX window (the last two chunks).  Prefix gathers only wait on chunk 3
# (lands ~257us, well before the encoder ends), so their ~7us serial gpsimd
# issues and executions all run during the all-gather tail; KPRE of them are
# emitted ahead of the msg loop.  Suffix-free tiles 0..NSFREE-1 (see the
# host permutation) keep the PE decoding through the final chunk's landing.
KPRE = 2
NSFREE = 3

_bf16 = ml_dtypes.bfloat16

_PROGRAM_CACHE = {}


def _build_program(S_PRE, S_SUF):
    """Build the SPMD Bass program.

    Every dst tile's message pass reads two gather windows: PREFIX (h_full
    rows < PFX, i.e. all all-gather chunks but the last) and SUFFIX (the
    final chunk's rows).  Both feed one contiguous PSUM accumulation group
    together with the SBUF self-loop block.
    """
    import concourse.bass as bass
    import concourse.bacc as bacc
    import concourse.mybir as mybir
    from concourse import library_config
    from concourse.tile import TileContext

    P = 128
    f32 = mybir.dt.float32
    f16 = mybir.dt.float16
    bf16 = mybir.dt.bfloat16
    NBLK_PRE = sum(S_PRE)
    NBLK = NBLK_PRE + sum(S_SUF)
    BOFF_PRE = [sum(S_PRE[:t]) for t in range(UT)]
    BOFF_SUF = [NBLK_PRE + sum(S_SUF[:t]) for t in range(UT)]

    nc = bacc.Bacc(
        "TRN2",
        target_bir_lowering=False,
        debug=False,
        num_devices=M,
        num_swdge_queues=4,
    )

    # x pre-transposed on host: row ut*128+p (item-in-chunk), col k*128+u
    x_d = nc.declare_dram_parameter("x", [UPAD, KC * P], bf16, isOutput=False)
    encw_d = nc.declare_dram_parameter("encw", [P, KC * HIDDEN], bf16, isOutput=False)
    decw_d = nc.declare_dram_parameter("decw", [P, 4 * N_ITEMS], bf16, isOutput=False)
    i16 = mybir.dt.int16
    # gather indices for dma_gather: idx j of tile t at column boff[t]*8 +
    # j//16, partition j%16, replicated 8x down the 128 partitions.
    si_d = nc.declare_dram_parameter("sidx", [P, NBLK * 8], i16, isOutput=False)
    wb_d = nc.declare_dram_parameter("wblk", [NBLK + UT, P, P], bf16, isOutput=False)
    rv_d = nc.declare_dram_parameter("rowvec", [4, UPAD], bf16, isOutput=False)
    out_d = nc.declare_dram_parameter("out", [UPC, N_ITEMS], f16, isOutput=True)

    # only the 2500 real rows travel through the all-gather.  Rows are 512
    # wide (dma_gather needs a 256B-multiple row stride); cols 500-511 are
    # never written — the garbage lands only in hmT rows that the rv DMA
    # overwrites (hidden 500+) or that carry zero decoder weight.
    HW = 512
    h_loc = nc.dram_tensor("h_loc", [UPC, HW], bf16)
    h_full = nc.dram_tensor("h_full", [M * UPC, HW], bf16, addr_space="Shared")
    CC_ROW_BOUNDS = [min(b * 128, UPC) for b in CC_TILE_BOUNDS]
    PFX = M * CC_ROW_BOUNDS[-3]  # prefix rows: chunks 0..3
    PFXB = M * CC_ROW_BOUNDS[-2]  # tile 3's wider prefix: chunks 0..4

    with TileContext(nc) as tc:
        with (
            tc.tile_pool(name="const", bufs=1) as cpool,
            tc.tile_pool(name="xin", bufs=2) as xpool,
            tc.tile_pool(name="hsb", bufs=UT) as hpool,
            # small bufs double as a THROTTLE: pre(t+k)'s gather reuses an
            # earlier tile's buffer, so it cannot execute until that tile's
            # msg consumed it — at most ~3 gathers' random reads ever collide
            # with the in-flight all-gather chunks (measured: unthrottled
            # gathers stretched a 35us chunk to 64us).
            tc.tile_pool(name="gpre", bufs=3) as prepool,
            tc.tile_pool(name="gsuf", bufs=2) as sufpool,
            tc.tile_pool(name="wbl", bufs=4 * max(S_PRE) + 10) as wpool,
            tc.tile_pool(name="hmt", bufs=4) as mpool,
            tc.tile_pool(name="pout", bufs=2) as opool,
            tc.tile_pool(name="ps_acc", bufs=3, space="PSUM") as ps_acc,
            tc.tile_pool(name="ps_dec", bufs=5, space="PSUM") as ps_dec,
        ):
            # dma_gather (InstDMAGatherAnt) lives in the gpsimd mlp library;
            # load it before any gpsimd instruction.
            nc.gpsimd.load_library(library_config.mlp)

            # encoder weights split into pieces so tile 0's matmuls start
            # after a few hundred KB (first pieces) instead of 7.5MB.
            enc_sb = cpool.tile([P, KC * HIDDEN], bf16, tag="encw")
            ENC_SPLIT = [6, 18, 32, KC]
            nc.sync.dma_start(
                out=enc_sb[:, : ENC_SPLIT[0] * HIDDEN],
                in_=encw_d[:, : ENC_SPLIT[0] * HIDDEN],
            )
            si_sb = cpool.tile([P, NBLK * 8], i16, tag="sidx")
            dec_sb = cpool.tile([P, 4 * N_ITEMS], bf16, tag="decw")

            # ---------------- Phase 1: encoder ----------------
            def load_x(ut):
                xb = xpool.tile([P, KC * P], bf16, tag="xb", name=f"xb{ut}")
                nc.sync.dma_start(out=xb[:], in_=x_d[ut * P : (ut + 1) * P, :])
                return xb

            hsbs = []
            xb_next = load_x(0)
            for ut in range(UT):
                xb = xb_next
                if ut == 0:
                    # interleave the remaining encoder-weight pieces with the
                    # x1 load so neither starves the tile-0/1 matmul chain.
                    nc.sync.dma_start(
                        out=enc_sb[:, ENC_SPLIT[0] * HIDDEN : ENC_SPLIT[1] * HIDDEN],
                        in_=encw_d[:, ENC_SPLIT[0] * HIDDEN : ENC_SPLIT[1] * HIDDEN],
                    )
                    xb_next = load_x(1)
                    e_lo = ENC_SPLIT[1]
                    for e_hi in ENC_SPLIT[2:]:
                        nc.sync.dma_start(
                            out=enc_sb[:, e_lo * HIDDEN : e_hi * HIDDEN],
                            in_=encw_d[:, e_lo * HIDDEN : e_hi * HIDDEN],
                        )
                        e_lo = e_hi
                elif ut + 1 < UT:
                    xb_next = load_x(ut + 1)
                h_ps = ps_acc.tile([P, 512], f32, tag="acc")
                for k in range(KC):
                    nc.tensor.matmul(
                        out=h_ps[:, :HIDDEN],
                        lhsT=xb[:, k * P : (k + 1) * P],
                        rhs=enc_sb[:, k * HIDDEN : (k + 1) * HIDDEN],
                        start=(k == 0),
                        stop=(k == KC - 1),
                    )
                # bufs=UT and exactly UT allocations of this tag: every tile
                # keeps its own SBUF-resident buffer for the whole kernel.
                hsb = hpool.tile([P, HPAD], bf16, tag="hsb")
                hsbs.append(hsb)
                nc.scalar.activation(
                    out=hsb[:, :HIDDEN],
                    in_=h_ps[:, :HIDDEN],
                    func=mybir.ActivationFunctionType.Sigmoid,
                )
                nc.vector.memset(hsb[:, HIDDEN:HPAD], 0.0)
                nr = min((ut + 1) * P, UPC) - ut * P  # 68 real rows on tile 19
                nc.sync.dma_start(
                    out=h_loc[ut * P : ut * P + nr, :HIDDEN], in_=hsb[:nr, :HIDDEN]
                )
                if ut == 2:
                    # deferred const loads: issued after the first x tiles so
                    # the encoder pipeline fills before they take bandwidth.
                    nc.sync.dma_start(out=si_sb[:], in_=si_d[:])
                elif ut in (4, 7, 10, 13):
                    # decoder weights in 1.5MB quarters spread across the
                    # encoder so no single load starves the x-tile stream.
                    j = (4, 7, 10, 13).index(ut)
                    nc.sync.dma_start(
                        out=dec_sb[:, j * N_ITEMS : (j + 1) * N_ITEMS],
                        in_=decw_d[:, j * N_ITEMS : (j + 1) * N_ITEMS],
                    )
                # ---- Phase 2 (interleaved): chunked all-gather ----
                if (ut + 1) in CC_TILE_BOUNDS:
                    j = CC_TILE_BOUNDS.index(ut + 1)
                    rlo = 0 if j == 0 else CC_ROW_BOUNDS[j - 1]
                    rhi = CC_ROW_BOUNDS[j]
                    nc.gpsimd.collective_compute(
                        "AllGather",
                        mybir.AluOpType.bypass,
                        replica_groups=[list(range(M))],
                        ins=[h_loc[rlo:rhi, :]],
                        outs=[h_full[M * rlo : M * rhi, :]],
                    )

            # ---------------- Phase 3: message passing + decoder ----------------
            # Software-pipelined: message matmuls of tile t are emitted before
            # decoder matmuls of tile t-1, so the PE stays busy while ACT
            # drains agg(t) into hmT(t).
            hmTs = [None] * UT
            GTW_PRE = max([1] + list(S_PRE))
            GTW_SUF = max([1] + list(S_SUF))
            pres = [None] * UT
            sufs = [None] * UT

            def gather(St, boff, lo, hi, q, pool, W):
                # ONE dma_gather fetches all St*128 source rows: out[p, s, :]
                # = h_full[lo + idx[s*128+p]].  The sliced source AP keeps the
                # dependency on just the all-gather chunks covering [lo, hi).
                gt_all = pool.tile([P, W, HW], bf16, tag="gt")
                nc.gpsimd.dma_gather(
                    gt_all[:, :St, :],
                    h_full[lo:hi, :],
                    si_sb[:, boff * 8 : (boff + St) * 8],
                    St * P,
                    St * P,
                    HW,
                    queue_num=q,
                )
                return gt_all

            def emit_pre(t):
                # prefix waits only on all-gather chunks 0..3 (0..4 for tile
                # 3's wider window), so its serial gpsimd issue overlaps the
                # collective tail.
                if S_PRE[t]:
                    pres[t] = gather(
                        S_PRE[t], BOFF_PRE[t], 0, PFXB if t == 3 else PFX,
                        t % 4, prepool, GTW_PRE,
                    )

            def emit_suf(t):
                # suffix waits on the final all-gather chunk
                if S_SUF[t]:
                    sufs[t] = gather(
                        S_SUF[t], BOFF_SUF[t], PFX, M * UPC, (t + 2) % 4,
                        sufpool, GTW_SUF,
                    )

            def load_wbs(boff, n):
                wbs = []
                for s in range(n):
                    wb = wpool.tile([P, P], bf16, tag="wb")
                    nc.sync.dma_start(out=wb[:], in_=wb_d[boff + s])
                    wbs.append(wb)
                return wbs

            def emit_msg_core(t, agg_ps):
                # drain agg psum -> hmT bf16 (+ bias/fill rows via DMA)
                hmT = mpool.tile([P, 512], bf16, tag="hmT")
                hmTs[t] = hmT
                # hidden unit 500 (chunk 3, row 122): decoder-bias unit
                # hidden unit 501 (chunk 3, row 123): row-mask fill unit
                # rows 124-125 are zero padding.  The ACT copy below skips
                # rows 122+ of chunk 3, so this DMA has no dependency on the
                # agg drain and can land during the message matmuls.
                nc.sync.dma_start(
                    out=hmT[122:126, 3 * P : 4 * P],
                    in_=rv_d[0:4, t * P : (t + 1) * P],
                )
                nc.scalar.activation(
                    out=hmT[0:126, 0 : 3 * P],
                    in_=agg_ps[0:126, 0 : 3 * P],
                    func=mybir.ActivationFunctionType.Copy,
                )
                nc.scalar.activation(
                    out=hmT[0:122, 3 * P : 4 * P],
                    in_=agg_ps[0:122, 3 * P : 4 * P],
                    func=mybir.ActivationFunctionType.Copy,
                )

            def emit_msg(t):
                agg_ps = ps_acc.tile([P, 512], f32, tag="acc")
                Sp, Ss = S_PRE[t], S_SUF[t]
                wbs = load_wbs(BOFF_PRE[t], Sp)
                wbs += load_wbs(BOFF_SUF[t], Ss)
                # self-loop block: this core's own h tile straight from SBUF
                # with a diagonal weight block (no gather, no DMA).
                wbself = wpool.tile([P, P], bf16, tag="wb")
                nc.sync.dma_start(out=wbself[:], in_=wb_d[NBLK + t])
                wbs.append(wbself)
                nblk = Sp + Ss  # + 1 self block
                # keep each PSUM sub-region's accumulation group contiguous:
                # interleaved start=True matmuls in one bank clobber each
                # other's accumulation state.
                for c in range(4):
                    for s in range(nblk + 1):
                        if s < Sp:
                            lhsT = pres[t][:, s, c * 126 : (c + 1) * 126]
                        elif s < nblk:
                            lhsT = sufs[t][:, s - Sp, c * 126 : (c + 1) * 126]
                        else:
                            lhsT = hsbs[t][:, c * 126 : (c + 1) * 126]
                        nc.tensor.matmul(
                            out=agg_ps[0:126, c * P : (c + 1) * P],
                            lhsT=lhsT,
                            rhs=wbs[s][:],
                            start=(s == 0),
                            stop=(s == nblk),
                        )
                emit_msg_core(t, agg_ps)

            def emit_dec(t):
                hmT = hmTs[t]
                nu = UPC - t * P if t == UT - 1 else P  # 68 on the last tile
                for half in range(2):
                    # batch 6 x 500-col chunks into one SBUF row-block so the
                    # output DMA moves contiguous 6KB rows.
                    psb = opool.tile([P, 3000], f16, tag="psb")
                    for nn in range(6):
                        n = half * 6 + nn
                        p_ps = ps_dec.tile([P, 512], f32, tag="pps")
                        for c in range(4):
                            nc.tensor.matmul(
                                out=p_ps[:, :500],
                                lhsT=hmT[0:126, c * P : (c + 1) * P],
                                rhs=dec_sb[0:126, c * N_ITEMS + n * 500 : c * N_ITEMS + (n + 1) * 500],
                                start=(c == 0),
                                stop=(c == 3),
                            )
                        nc.vector.tensor_scalar(
                            out=psb[:, nn * 500 : (nn + 1) * 500],
                            in0=p_ps[:, :500],
                            scalar1=R_MAX,
                            scalar2=R_MIN,
                            op0=mybir.AluOpType.min,
                            op1=mybir.AluOpType.max,
                        )
                        if t == UT - 1 and nn % 2 == 1:
                            # last tile: drain in 1000-col pieces right behind
                            # the DVE so the kernel tail is one small DMA.
                            nc.sync.dma_start(
                                out=out_d[
                                    t * P : t * P + nu,
                                    half * 3000 + (nn - 1) * 500 : half * 3000 + (nn + 1) * 500,
                                ],
                                in_=psb[:nu, (nn - 1) * 500 : (nn + 1) * 500],
                            )
                    if t != UT - 1:
                        nc.sync.dma_start(
                            out=out_d[t * P : t * P + nu, half * 3000 : (half + 1) * 3000],
                            in_=psb[:nu, :],
                        )

            # KPRE tiles' prefix gathers are emitted ahead so their serial
            # gpsimd issues overlap the all-gather tail; suffix gathers come
            # after ALL primed prefixes (a suffix waits on the final chunk,
            # and gpsimd executes in order — a waiting suffix must not block
            # prefix issues).  Then the usual msg(t) / dec(t-1) interleave
            # with gathers staying KPRE tiles ahead.
            for t in range(KPRE):
                emit_pre(t)
            for t in range(KPRE):
                emit_suf(t)
            for t in range(UT):
                emit_msg(t)
                if t + KPRE < UT:
                    emit_pre(t + KPRE)
                    emit_suf(t + KPRE)
                if t > 0:
                    emit_dec(t - 1)
            emit_dec(UT - 1)

    nc.finalize()
    return nc


def _prep_host(x, edge_index, edge_weight, ft_n0, ft_n1, fill_const,
               enc_w, enc_b, dec_w, dec_b, conv_w):
    """All host-side preprocessing: sharding, weight prep, edge packing."""
    x = np.asarray(x, np.float32)
    src = np.asarray(edge_index[0], np.int64)
    dst = np.asarray(edge_index[1], np.int64)
    w = np.asarray(edge_weight, np.float32)
    ft_n0 = np.asarray(ft_n0)
    ft_n1 = np.asarray(ft_n1)
    fill = float(np.asarray(fill_const))
    conv = float(np.asarray(conv_w))
    enc_w = np.asarray(enc_w, np.float32)
    enc_b = np.asarray(enc_b, np.float32)
    dec_w = np.asarray(dec_w, np.float32)
    dec_b = np.asarray(dec_b, np.float32)

    # ---- user permutation: early dst tiles never wait on late chunks ----
    # Row order within a core is ours to choose (the output rows are
    # un-permuted on the host afterwards, costing no device time).  Swap
    # users so that:
    #   R5 rows [2304, 2500) (the final all-gather chunk) source no edges
    #      into dst tiles 0..3 of any core;
    #   R4 rows [1920, 2304) (chunk 4) source no edges into tiles 0..2.
    # Tiles 0-2 then complete from prefix+self alone, and tile 3's gather
    # window only extends through chunk 4 — nothing early ever waits on the
    # final chunk's ~17us fixed-latency tail.  Strict for tiles 0-1,
    # best-effort beyond (leftovers degrade gracefully).
    SFX_LO = CC_TILE_BOUNDS[-3] * 128  # 1920: suffix rows (chunk 4) start
    SFX5_LO = CC_TILE_BOUNDS[-2] * 128  # 2304: final chunk's rows start
    src0 = np.asarray(edge_index[0], np.int64)
    dst0 = np.asarray(edge_index[1], np.int64)
    dloc0 = dst0 % UPC

    def _ban(k):  # users with an edge into dst rows < k*128 of any core
        b = np.zeros(N_USERS, bool)
        b[src0[dloc0 < k * 128]] = True
        return b

    ban2, ban3, ban4 = _ban(2), _ban(3), _ban(4)
    tier2 = ban3 & ~ban2
    tier3 = ban4 & ~ban3
    perm = np.arange(N_USERS)  # position -> user
    E0 = 4 * 128  # positions below this are never moved (they define bans)
    for c in range(M):
        u0 = c * UPC
        pool_ok = np.ones(UPC, bool)  # per-position: still swappable

        def swap_region(rlo, rhi, ban_full, tiers):
            pos = np.arange(u0 + rlo, u0 + rhi)
            u = perm[pos]
            need = np.concatenate([pos[t[u]] for t in tiers])
            mid = np.arange(u0 + E0, u0 + SFX_LO)
            mid = mid[pool_ok[mid - u0] & ~ban_full[perm[mid]]]
            k = min(len(need), len(mid))
            need, mid = need[:k], mid[:k]
            a = perm[need].copy()
            perm[need] = perm[mid]
            perm[mid] = a
            pool_ok[mid - u0] = False

        swap_region(SFX5_LO, UPC, ban4, [ban2, tier2, tier3])
        swap_region(SFX_LO, SFX5_LO, ban3, [ban2, tier2])
    pos_of = np.empty(N_USERS, np.int64)
    pos_of[perm] = np.arange(N_USERS)

    x = x[perm]
    ft_n0 = np.asarray(ft_n0)[perm]
    src = pos_of[src]
    dst = pos_of[dst]

    rowmask = ft_n0 == 0  # rows forced to fill
    colmask = ft_n1 == 0  # cols forced to fill

    # ---- x per core, transposed to item-major tiles on host ----
    # layout: [UT, 128 (item-in-chunk p), KC, 128 (user u)] so each user
    # tile is one contiguous [128, KC*128] bf16 DMA and lhsT chunks are
    # direct column slices.
    xp = np.zeros((M, UPAD, IPAD), np.float32)
    xp[:, :UPC, :N_ITEMS] = x.reshape(M, UPC, N_ITEMS)
    xp[:, :, N_ITEMS] = 1.0  # encoder-bias input column
    xt_host = np.ascontiguousarray(
        xp.reshape(M, UT, 128, KC, 128).transpose(0, 1, 4, 3, 2)
    ).astype(_bf16).reshape(M, UPAD, KC * 128)

    # ---- encoder weights: [6016, 500] -> [128, 47*500] chunk-major ----
    ewp = np.zeros((IPAD, HIDDEN), np.float32)
    ewp[:N_ITEMS] = enc_w.T
    ewp[N_ITEMS] = enc_b
    enc_host = np.ascontiguousarray(
        ewp.reshape(KC, 128, HIDDEN).transpose(1, 0, 2).reshape(128, KC * HIDDEN)
    ).astype(_bf16)

    # ---- decoder weights with baked column mask / bias / fill units ----
    dw = dec_w.T.copy()  # [500, 6000]
    dw[:, colmask] = 0.0
    hp = np.zeros((HPAD, N_ITEMS), np.float32)
    hp[:HIDDEN] = dw
    hp[HIDDEN] = np.where(colmask, fill, dec_b)  # bias unit
    hp[HIDDEN + 1] = fill  # row-mask fill unit (all cols)
    dec_host = np.zeros((128, 4, N_ITEMS), np.float32)
    dec_host[:126] = hp.reshape(4, 126, N_ITEMS).transpose(1, 0, 2)
    dec_host = np.ascontiguousarray(dec_host.reshape(128, 4 * N_ITEMS)).astype(_bf16)

    # ---- edges: filter masked dst, fold conv_w ----
    keep = ~rowmask[dst]
    src_a = src[keep]
    dst_a = dst[keep]
    w_a = w[keep] * conv

    order = np.argsort(dst_a, kind="stable")
    src_a, dst_a, w_a = src_a[order], dst_a[order], w_a[order]

    core = dst_a // UPC
    ldst = dst_a - core * UPC
    tile_g = core * UT + ldst // 128  # global tile id (sorted ascending)
    din = (ldst % 128).astype(np.int64)

    # gather index into the PADDED all-gathered h table.
    # h_full layout after the uneven chunked all-gather: chunk j covers local
    # rows [lo_j*128, hi_j*128) of every core, concatenated core-major:
    # row = off_j + core * crows_j + (local - lo_j*128)
    src_core = src_a // UPC
    src_loc = src_a % UPC
    bounds_rows = np.array([min(b * 128, UPC) for b in CC_TILE_BOUNDS])
    starts_rows = np.concatenate([[0], bounds_rows[:-1]])
    crows = bounds_rows - starts_rows
    offs = np.concatenate([[0], np.cumsum(M * crows)[:-1]])
    cjs = np.searchsorted(bounds_rows, src_loc, side="right")
    gsrc_e = (
        offs[cjs] + src_core * crows[cjs] + (src_loc - starts_rows[cjs])
    ).astype(np.int64)

    # per-(tile, window) block quotas (max over cores, so the SPMD program is
    # identical on every core).  Every tile is split into a PREFIX window
    # (sources in h_full rows written by chunks 0..3, idx as-is) and a
    # SUFFIX window (last two chunks' rows, idx rebased) so the prefix
    # gathers only wait on all-gather chunk 3.  Tile 3's prefix window
    # extends through chunk 4 (the permutation keeps its sources out of the
    # final chunk), so its single gather waits only on chunk 4.
    PFX = M * int(bounds_rows[-3])
    PFXB = M * int(bounds_rows[-2])
    t_of_edge = tile_g % UT
    thr = np.full(UT, PFX, np.int64)
    thr[3] = PFXB
    in_sfx = gsrc_e >= thr[t_of_edge]

    def pack(sel, rebase, min1_from=None):
        """Pack selected edges into per-tile 128-edge blocks.

        dma_gather index layout: idx j of tile t at column boff[t]*8 + j//16,
        partition j%16, replicated 8x down the 128 partitions.  Padding uses
        index 0 (gathers a real row, multiplied by weight 0).
        """
        tg = tile_g[sel]
        gi_all = gsrc_e[sel] - rebase
        dn = din[sel]
        ww = w_a[sel]
        cnt = np.bincount(tg, minlength=M * UT).reshape(M, UT)
        S_t = np.ceil(cnt.max(axis=0) / 128).astype(np.int64)
        if min1_from is not None:
            S_t[min1_from:] = np.maximum(1, S_t[min1_from:])
        boff = np.concatenate([[0], np.cumsum(S_t)[:-1]])
        nblk = int(S_t.sum())
        si_h = np.zeros((M, 128, nblk * 8), np.int16)
        wb_h = np.zeros((M, nblk, 128, 128), np.float32)
        starts = np.zeros(M * UT + 1, np.int64)
        np.cumsum(cnt.reshape(-1), out=starts[1:])
        for g in range(M * UT):
            c, t = divmod(g, UT)
            St = int(S_t[t])
            if St == 0:
                continue
            n = int(cnt[c, t])
            sl = slice(starts[g], starts[g] + n)
            cap = St * 128
            gi = np.zeros(cap, np.int64)
            wi = np.zeros(cap, np.float32)
            di = np.zeros(cap, np.int64)
            gi[:n] = gi_all[sl]
            wi[:n] = ww[sl]
            di[:n] = dn[sl]
            b0 = int(boff[t])
            wrap = gi.astype(np.int16).reshape(-1, 16).T  # [16, S*8]
            si_h[c, :, b0 * 8 : (b0 + St) * 8] = np.tile(wrap, (8, 1))
            for q in range(St):
                blk = slice(q * 128, (q + 1) * 128)
                wb_h[c, b0 + q][np.arange(128), di[blk]] = wi[blk]
        return S_t, si_h, wb_h

    SP_f, si_P, wb_P = pack(~in_sfx, 0)
    SS_f, si_S, wb_S = pack(in_sfx, PFX)
    S_PRE = tuple(int(v) for v in SP_f)
    S_SUF = tuple(int(v) for v in SS_f)
    si_host = np.concatenate([si_P, si_S], axis=2)
    wblk_host = np.concatenate([wb_P, wb_S], axis=1)
    # diagonal self-loop weight blocks, appended after the gather blocks:
    # block NBLK + t applies (1-conv)*live(d) to the SBUF h tile t.
    lv = np.zeros((M, UPAD), np.float32)
    lv[:, :UPC] = (~rowmask).reshape(M, UPC).astype(np.float32) * (1.0 - conv)
    wself = np.zeros((M, UT, 128, 128), np.float32)
    di128 = np.arange(128)
    for t in range(UT):
        wself[:, t, di128, di128] = lv[:, t * 128 : (t + 1) * 128]
    wblk_host = np.concatenate([wblk_host, wself], axis=1).astype(_bf16)

    # ---- row vectors: bias-unit coeff and row-mask coeff per padded user
    # (rows 2-3 are zero fillers for hmT pad rows 124-125) ----
    rv = np.zeros((M, 4, UPAD), np.float32)
    rm = rowmask.reshape(M, UPC)
    rv[:, 0, :UPC] = (~rm).astype(np.float32)  # bias unit on for live rows
    rv[:, 1, :UPC] = rm.astype(np.float32)     # fill unit on for masked rows
    rv_host = rv.astype(_bf16)

    in_maps = []
    for c in range(M):
        in_maps.append(
            {
                "x": xt_host[c],
                "encw": enc_host,
                "decw": dec_host,
                "sidx": si_host[c],
                "wblk": wblk_host[c],
                "rowvec": rv_host[c],
            }
        )
    return S_PRE, S_SUF, pos_of, in_maps


def _install_ntff_hook_shim():
    """The agent image's antenv lacks axon_hooks; synthesize it so
    run_bass_kernel_spmd(trace=True) can capture NTFF profiles."""
    import types

    if "antenv.axon_hooks" in sys.modules:
        return
    try:
        from trn_agent_boot.trn_boot import _ntff_profile_via_ctypes
    except ImportError:
        return
    hook = _ntff_profile_via_ctypes("/opt/axon/libaxon_pjrt.so")
    mod = types.ModuleType("antenv.axon_hooks")
    mod._hook = hook
    mod.set_axon_ntff_profile_hook = lambda h: setattr(mod, "_hook", h)
    mod.get_axon_ntff_profile_hook = lambda: mod._hook
    sys.modules["antenv.axon_hooks"] = mod
    try:
        import antenv

        antenv.axon_hooks = mod
    except ImportError:
        pass


LAST_EXEC_NS = None
LAST_RESULTS = None


def kernel(x, edge_index, edge_weight, ft_n0, ft_n1, fill_const,
           enc_w, enc_b, dec_w, dec_b, conv_w):
    global LAST_EXEC_NS, LAST_RESULTS
    from concourse.bass_utils import run_bass_kernel_spmd

    S_PRE, S_SUF, pos_of, in_maps = _prep_host(
        x, edge_index, edge_weight, ft_n0, ft_n1, fill_const,
        enc_w, enc_b, dec_w, dec_b, conv_w,
    )

    key = (S_PRE, S_SUF)
    if key not in _PROGRAM_CACHE:
        _PROGRAM_CACHE[key] = _build_program(S_PRE, S_SUF)
    nc = _PROGRAM_CACHE[key]

    trace = os.environ.get("KERNEL_TRACE", "0") == "1"
    tmpdir = os.environ.get("KERNEL_TRACE_DIR") or None
    if trace:
        _install_ntff_hook_shim()
    res = run_bass_kernel_spmd(
        nc,
        in_maps,
        core_ids=list(range(M)),
        trace=trace,
        tmpdir=tmpdir,
    )
    LAST_EXEC_NS = res.exec_time_ns
    LAST_RESULTS = res
    out = np.concatenate([res.results[c]["out"] for c in range(M)], axis=0)
    # rows come back in permuted (position) order; row for user u is at
    # position pos_of[u]
    return np.ascontiguousarray(out[pos_of].astype(np.float32))

